# revision 2
# baseline (speedup 1.0000x reference)
"""Trainium2 Bass kernel for the MiniBatchAUC pairwise surrogate loss.

Math: with s = sigmoid(logits), pos/neg the 0/1 target masks,
    loss_sum = sum_{i in P, j in N} (1 - s_i + s_j)^2
factorizes exactly (expand the square; the double sum separates):
    loss_sum = n_neg * Sp2 + 2 * Sp1 * Sn1 + n_pos * Sn2
      Sp1 = sum_P (1-s),  Sp2 = sum_P (1-s)^2,
      Sn1 = sum_N s,      Sn2 = sum_N s^2.
The O(N^2) pairwise matrix is never materialized: each core reduces its
2048-element shard to partial sums; the host all-reduces the per-core
partials and applies the closed form.

Per-core device program (SPMD, identical on all 8 cores):
  - one DMA in: [128, 33] f32 tile = logits(16 cols) | targets(16) | ones(1)
  - ACT: a = sigmoid(-L) (== 1-s), s = sigmoid(L), u = 1-T (fused accum
    gives the per-partition negative count)
  - DVE: 4 plain multiplies T*a, (T*a)*a, u*s, (u*s)*s into one wide tile
    (tensor_tensor_reduce is broken on this terminal - device crash)
  - PE: one matmul with the ones column partition-reduces [128,65] -> [1,65]
  - DMA out: 260 bytes; host sums the 16-wide groups and combines cores
"""

import numpy as np

try:
    import concourse.bass as bass
except ImportError:  # concourse ships in the container, not on sys.path
    import sys

    sys.path.insert(0, "/opt/trn_rl_repo")
    import concourse.bass as bass

import concourse.tile as tile
from concourse import bacc, mybir
from concourse import bass_utils

N = 16384
NCORES = 8
SHARD = N // NCORES  # 2048 elements per core
P = 128  # SBUF partitions
F = SHARD // P  # 16 free elements per partition
W = 2 * F + 1  # logits | targets | ones
OUTW = 4 * F + 1  # 4 product groups + neg-count column

f32 = mybir.dt.float32

_CACHE: dict = {}


def _build():
    nc = bacc.Bacc(
        "TRN2",
        target_bir_lowering=False,
        debug=False,
        enable_asserts=False,
        num_devices=NCORES,
    )
    x_dram = nc.dram_tensor("x", [P, W], f32, kind="ExternalInput").ap()
    o_dram = nc.dram_tensor("o", [1, OUTW], f32, kind="ExternalOutput").ap()

    Sig = mybir.ActivationFunctionType.Sigmoid
    Copy = mybir.ActivationFunctionType.Copy

    with tile.TileContext(nc) as tc:
        with (
            tc.tile_pool(name="sbuf", bufs=1) as pool,
            tc.tile_pool(name="psum", bufs=1, space="PSUM") as psum_pool,
        ):
            x = pool.tile([P, W], f32)
            nc.sync.dma_start(x[:], x_dram)
            L = x[:, 0:F]
            T = x[:, F : 2 * F]
            ones = x[:, 2 * F : 2 * F + 1]

            a = pool.tile([P, F], f32)  # 1 - s, computed as sigmoid(-L)
            s = pool.tile([P, F], f32)
            u = pool.tile([P, F], f32)  # 1 - T (negative mask)
            w = pool.tile([P, OUTW], f32)
            ta = w[:, 0:F]
            ta2 = w[:, F : 2 * F]
            us = w[:, 2 * F : 3 * F]
            us2 = w[:, 3 * F : 4 * F]

            nc.scalar.activation(a[:], L, Sig, scale=-1.0)
            nc.scalar.activation(s[:], L, Sig)
            # u = 1 - T; fused accum gives the per-partition negative count
            nc.scalar.activation(
                u[:], T, Copy, bias=1.0, scale=-1.0, accum_out=w[:, 4 * F : 4 * F + 1]
            )

            nc.vector.tensor_mul(ta, T, a[:])  # T*(1-s)
            nc.vector.tensor_mul(ta2, ta, a[:])  # T*(1-s)^2
            nc.vector.tensor_mul(us, u[:], s[:])  # (1-T)*s
            nc.vector.tensor_mul(us2, us, s[:])  # (1-T)*s^2

            acc = psum_pool.tile([1, OUTW], f32)
            nc.tensor.matmul(acc[:], ones, w[:], start=True, stop=True)
            out_sb = pool.tile([1, OUTW], f32)
            nc.scalar.copy(out_sb[:], acc[:])
            nc.sync.dma_start(o_dram, out_sb[:])

    nc.compile()
    return nc


def _get_nc():
    if "nc" not in _CACHE:
        _CACHE["nc"] = _build()
    return _CACHE["nc"]


def make_in_maps(logits: np.ndarray, targets: np.ndarray) -> list[dict]:
    logits = np.ascontiguousarray(logits, dtype=np.float32)
    t32 = np.asarray(targets).astype(np.float32)  # values are 0/1; lossless
    in_maps = []
    for k in range(NCORES):
        sl = slice(k * SHARD, (k + 1) * SHARD)
        xk = np.empty((P, W), np.float32)
        xk[:, 0:F] = logits[sl].reshape(P, F)
        xk[:, F : 2 * F] = t32[sl].reshape(P, F)
        xk[:, 2 * F :] = 1.0
        in_maps.append({"x": xk})
    return in_maps


def combine(outs: np.ndarray) -> np.ndarray:
    """All-reduce the [NCORES, OUTW] partials and apply the closed form."""
    tot = outs.astype(np.float64).sum(axis=0)
    n_neg = tot[4 * F]
    n_pos = float(N) - n_neg
    sp1 = tot[0:F].sum()
    sp2 = tot[F : 2 * F].sum()
    sn1 = tot[2 * F : 3 * F].sum()
    sn2 = tot[3 * F : 4 * F].sum()
    loss = (n_neg * sp2 + 2.0 * sp1 * sn1 + n_pos * sn2) / (n_pos * n_neg)
    return np.array(loss, dtype=np.float32)


def kernel(logits: np.ndarray, targets: np.ndarray, **run_kwargs):
    nc = _get_nc()
    res = bass_utils.run_bass_kernel_spmd(
        nc, make_in_maps(logits, targets), core_ids=list(range(NCORES)), **run_kwargs
    )
    outs = np.stack([r["o"][0] for r in res.results])  # [8, OUTW]
    out = combine(outs)
    _CACHE["last_results"] = res
    return out


# revision 3
# speedup vs baseline: 1130.4092x; 1130.4092x over previous
"""Trainium2 Bass kernel for the MiniBatchAUC pairwise surrogate loss.

Math: with s = sigmoid(logits), pos/neg the 0/1 target masks,
    loss_sum = sum_{i in P, j in N} (1 - s_i + s_j)^2
factorizes exactly (expand the square; the double sum separates):
    loss_sum = n_neg * Sp2 + 2 * Sp1 * Sn1 + n_pos * Sn2
      Sp1 = sum_P (1-s),  Sp2 = sum_P (1-s)^2,
      Sn1 = sum_N s,      Sn2 = sum_N s^2,
and with c = sum T, m1 = sum T*s, m2 = sum T*s^2, g1 = sum s, g2 = sum s^2:
      Sp1 = c - m1, Sp2 = c - 2*m1 + m2, Sn1 = g1 - m1, Sn2 = g2 - m2.
So the O(N^2) pairwise matrix is never materialized: each core reduces its
2048-element shard to 5 per-partition partial sums; the host all-reduces
the per-core partials and applies the closed form.

Per-core device program (SPMD, identical on all 8 cores), 10 instructions:
  - one DMA in: [128, 32] f32 tile = logits(16 cols) | targets(16)
  - ACT: s = sigmoid(L) (fused accum -> per-partition sum s),
         count = Copy(T) (fused accum -> per-partition sum T)
  - DVE: s*s, T*s, (T*s)*s multiplies + reduce_sum of each
    (tensor_tensor_reduce crashes this terminal's runtime; ACT Square in the
     s -> s2 chain is slower than overlapping the multiply on DVE)
  - one DMA out: the [128, 5] per-partition partials (2.5 KB)
No PE/PSUM involvement - the partition reduction is part of the host-side
all-reduce of partials (TimelineSim: 6794 ns vs 7537 ns with an
on-device ones-matmul partition reduction).
"""

import numpy as np

try:
    import concourse.bass as bass
except ImportError:  # concourse ships in the container, not on sys.path
    import sys

    sys.path.insert(0, "/opt/trn_rl_repo")
    import concourse.bass as bass

import concourse.tile as tile
from concourse import bacc, mybir
from concourse import bass_utils

N = 16384
NCORES = 8
SHARD = N // NCORES  # 2048 elements per core
P = 128  # SBUF partitions
F = SHARD // P  # 16 free elements per partition

f32 = mybir.dt.float32

_CACHE: dict = {}


def _build():
    nc = bacc.Bacc(
        "TRN2",
        target_bir_lowering=False,
        debug=False,
        enable_asserts=False,
        num_devices=NCORES,
    )
    x_dram = nc.dram_tensor("x", [P, 2 * F], f32, kind="ExternalInput").ap()
    o_dram = nc.dram_tensor("o", [P, 5], f32, kind="ExternalOutput").ap()

    Sig = mybir.ActivationFunctionType.Sigmoid
    Copy = mybir.ActivationFunctionType.Copy
    X = mybir.AxisListType.X

    with tile.TileContext(nc) as tc:
        with tc.tile_pool(name="sbuf", bufs=1) as pool:
            x = pool.tile([P, 2 * F], f32)
            nc.sync.dma_start(x[:], x_dram)
            L = x[:, 0:F]
            T = x[:, F : 2 * F]

            s = pool.tile([P, F], f32)
            s2 = pool.tile([P, F], f32)
            tcnt = pool.tile([P, F], f32)
            ts = pool.tile([P, F], f32)
            ts2 = pool.tile([P, F], f32)
            r = pool.tile([P, 5], f32)  # g1 | g2 | c | m1 | m2 per partition

            nc.scalar.activation(s[:], L, Sig, accum_out=r[:, 0:1])
            nc.vector.tensor_mul(s2[:], s[:], s[:])
            nc.vector.reduce_sum(r[:, 1:2], s2[:], axis=X)
            nc.scalar.activation(tcnt[:], T, Copy, accum_out=r[:, 2:3])
            nc.vector.tensor_mul(ts[:], T, s[:])
            nc.vector.tensor_mul(ts2[:], ts[:], s[:])
            nc.vector.reduce_sum(r[:, 3:4], ts[:], axis=X)
            nc.vector.reduce_sum(r[:, 4:5], ts2[:], axis=X)
            nc.sync.dma_start(o_dram, r[:])

    nc.compile()
    return nc


def _get_nc():
    if "nc" not in _CACHE:
        _CACHE["nc"] = _build()
    return _CACHE["nc"]


def make_in_maps(logits: np.ndarray, targets: np.ndarray) -> list[dict]:
    logits = np.ascontiguousarray(logits, dtype=np.float32)
    t32 = np.asarray(targets).astype(np.float32)  # values are 0/1; lossless
    in_maps = []
    for k in range(NCORES):
        sl = slice(k * SHARD, (k + 1) * SHARD)
        xk = np.empty((P, 2 * F), np.float32)
        xk[:, 0:F] = logits[sl].reshape(P, F)
        xk[:, F : 2 * F] = t32[sl].reshape(P, F)
        in_maps.append({"x": xk})
    return in_maps


def combine(outs: np.ndarray) -> np.ndarray:
    """All-reduce the [NCORES, P, 5] partials and apply the closed form."""
    tot = outs.astype(np.float64).sum(axis=(0, 1))
    g1, g2, c, m1, m2 = tot
    n_pos = c
    n_neg = float(N) - c
    sp1 = c - m1
    sp2 = c - 2.0 * m1 + m2
    sn1 = g1 - m1
    sn2 = g2 - m2
    loss = (n_neg * sp2 + 2.0 * sp1 * sn1 + n_pos * sn2) / (n_pos * n_neg)
    return np.array(loss, dtype=np.float32)


def kernel(logits: np.ndarray, targets: np.ndarray, **run_kwargs):
    nc = _get_nc()
    res = bass_utils.run_bass_kernel_spmd(
        nc, make_in_maps(logits, targets), core_ids=list(range(NCORES)), **run_kwargs
    )
    outs = np.stack([r["o"] for r in res.results])  # [8, 128, 5]
    out = combine(outs)
    _CACHE["last_results"] = res
    return out


# revision 5
# speedup vs baseline: 1165.5790x; 1.0311x over previous
"""Trainium2 Bass kernel for the MiniBatchAUC pairwise surrogate loss.

Math: with s = sigmoid(logits), pos/neg the 0/1 target masks,
    loss_sum = sum_{i in P, j in N} (1 - s_i + s_j)^2
factorizes exactly (expand the square; the double sum separates):
    loss_sum = n_neg * Sp2 + 2 * Sp1 * Sn1 + n_pos * Sn2
      Sp1 = sum_P (1-s),  Sp2 = sum_P (1-s)^2,
      Sn1 = sum_N s,      Sn2 = sum_N s^2,
and with c = sum T, m1 = sum T*s, m2 = sum T*s^2, g1 = sum s, g2 = sum s^2:
      Sp1 = c - m1, Sp2 = c - 2*m1 + m2, Sn1 = g1 - m1, Sn2 = g2 - m2.
So the O(N^2) pairwise matrix is never materialized: each core reduces its
2048-element shard to 5 per-partition partial sums; the host all-reduces
the per-core partials and applies the closed form.

Per-core device program (SPMD, identical on all 8 cores), 10 instructions:
  - one DMA in: [128, 32] f32 tile = logits(16 cols) | targets(16)
  - ACT: s = sigmoid(L) (fused accum -> per-partition sum s),
         count = Copy(T) (fused accum -> per-partition sum T)
  - DVE: s*s, T*s, (T*s)*s multiplies + reduce_sum of each
    (tensor_tensor_reduce crashes this terminal's runtime; ACT Square in the
     s -> s2 chain is slower than overlapping the multiply on DVE)
  - one DMA out: the [128, 5] per-partition partials (2.5 KB)
No PE/PSUM involvement - the partition reduction is part of the host-side
all-reduce of partials (TimelineSim: 6794 ns vs 7537 ns with an
on-device ones-matmul partition reduction).
"""

import numpy as np

try:
    import concourse.bass as bass
except ImportError:  # concourse ships in the container, not on sys.path
    import sys

    sys.path.insert(0, "/opt/trn_rl_repo")
    import concourse.bass as bass

import concourse.tile as tile
from concourse import bacc, mybir
from concourse import bass_utils

N = 16384
NCORES = 8
SHARD = N // NCORES  # 2048 elements per core
P = 128  # SBUF partitions
F = SHARD // P  # 16 free elements per partition

f32 = mybir.dt.float32

_CACHE: dict = {}


def _build():
    nc = bacc.Bacc(
        "TRN2",
        target_bir_lowering=False,
        debug=False,
        enable_asserts=False,
        num_devices=NCORES,
    )
    x_dram = nc.dram_tensor("x", [P, 2 * F], f32, kind="ExternalInput").ap()
    o_dram = nc.dram_tensor("o", [P, 5], f32, kind="ExternalOutput").ap()

    Sig = mybir.ActivationFunctionType.Sigmoid
    Copy = mybir.ActivationFunctionType.Copy
    X = mybir.AxisListType.X

    # Raw bacc with manual semaphores: no TileContext, so the Tile exit
    # drain + EVSEM butterfly never enters the program.
    with (
        nc.sbuf_tensor([P, 2 * F], f32) as x,
        nc.sbuf_tensor([P, F], f32) as s,
        nc.sbuf_tensor([P, F], f32) as s2,
        nc.sbuf_tensor([P, F], f32) as tcnt,
        nc.sbuf_tensor([P, F], f32) as ts,
        nc.sbuf_tensor([P, F], f32) as ts2,
        nc.sbuf_tensor([P, 5], f32) as r,  # g1 | g2 | c | m1 | m2
        nc.semaphore() as dsem,
        nc.semaphore() as asem,
        nc.semaphore() as vsem,
        nc.semaphore() as osem,
        nc.Block() as block,
    ):
        L = x[:, 0:F]
        T = x[:, F : 2 * F]

        @block.sync
        def _(sync):
            sync.dma_start(x[:], x_dram).then_inc(dsem, 16)
            sync.wait_ge(asem, 2)  # both ACT accums landed in r
            sync.wait_ge(vsem, 6)  # all DVE muls + reduces landed in r
            sync.dma_start(o_dram, r[:]).then_inc(osem, 16)
            sync.wait_ge(osem, 16)  # out-DMA complete before program end

        @block.scalar
        def _(scalar):
            scalar.wait_ge(dsem, 16)
            nc.scalar.activation(s[:], L, Sig, accum_out=r[:, 0:1]).then_inc(asem, 1)
            nc.scalar.activation(tcnt[:], T, Copy, accum_out=r[:, 2:3]).then_inc(
                asem, 1
            )

        @block.vector
        def _(vector):
            # Deep engine pipelines: same-engine RAW hazards need sem chains
            # (the race detector rejects back-to-back dependent DVE ops).
            vector.wait_ge(dsem, 16)  # T in SBUF
            vector.wait_ge(asem, 1)  # s written
            nc.vector.tensor_mul(ts[:], T, s[:]).then_inc(vsem, 1)
            nc.vector.tensor_mul(s2[:], s[:], s[:]).then_inc(vsem, 1)
            vector.wait_ge(vsem, 1)  # ts retired
            nc.vector.tensor_mul(ts2[:], ts[:], s[:]).then_inc(vsem, 1)
            nc.vector.reduce_sum(r[:, 3:4], ts[:], axis=X).then_inc(vsem, 1)
            vector.wait_ge(vsem, 2)  # s2 retired
            nc.vector.reduce_sum(r[:, 1:2], s2[:], axis=X).then_inc(vsem, 1)
            vector.wait_ge(vsem, 3)  # ts2 retired
            nc.vector.reduce_sum(r[:, 4:5], ts2[:], axis=X).then_inc(vsem, 1)

    nc.compile()
    return nc


def _get_nc():
    if "nc" not in _CACHE:
        _CACHE["nc"] = _build()
    return _CACHE["nc"]


def make_in_maps(logits: np.ndarray, targets: np.ndarray) -> list[dict]:
    logits = np.ascontiguousarray(logits, dtype=np.float32)
    t32 = np.asarray(targets).astype(np.float32)  # values are 0/1; lossless
    in_maps = []
    for k in range(NCORES):
        sl = slice(k * SHARD, (k + 1) * SHARD)
        xk = np.empty((P, 2 * F), np.float32)
        xk[:, 0:F] = logits[sl].reshape(P, F)
        xk[:, F : 2 * F] = t32[sl].reshape(P, F)
        in_maps.append({"x": xk})
    return in_maps


def combine(outs: np.ndarray) -> np.ndarray:
    """All-reduce the [NCORES, P, 5] partials and apply the closed form."""
    tot = outs.astype(np.float64).sum(axis=(0, 1))
    g1, g2, c, m1, m2 = tot
    n_pos = c
    n_neg = float(N) - c
    sp1 = c - m1
    sp2 = c - 2.0 * m1 + m2
    sn1 = g1 - m1
    sn2 = g2 - m2
    loss = (n_neg * sp2 + 2.0 * sp1 * sn1 + n_pos * sn2) / (n_pos * n_neg)
    return np.array(loss, dtype=np.float32)


def kernel(logits: np.ndarray, targets: np.ndarray, **run_kwargs):
    nc = _get_nc()
    res = bass_utils.run_bass_kernel_spmd(
        nc, make_in_maps(logits, targets), core_ids=list(range(NCORES)), **run_kwargs
    )
    outs = np.stack([r["o"] for r in res.results])  # [8, 128, 5]
    out = combine(outs)
    _CACHE["last_results"] = res
    return out
